# revision 1
# baseline (speedup 1.0000x reference)
"""Trainium2 Bass kernel for nn_HardestContrastiveLoss.

Strategy (1D row-parallel cdist, per sharding hint):
  - Host: gather the selected correspondences (pure indexing/layout), build
    transposed operand blocks, shard 8192 selected rows as 1024 rows/core.
  - Device (per core, identical program, different data):
      * scale gathered src feats by -2, rigid-transform gathered src points
        (rot/trans via a small matmul + fused add/scale), square + ones-matmul
        for the |.|^2 rows -> extended matmul operands
      * feats distance^2 matrix [1024, 8192] and pts distance^2 matrix as
        single matmuls with extended vectors [-2a, 1, |a|^2] . [b, |b|^2, 1]
      * masks from pts-d2 on ScalarE: t = relu(S*(c - pd2)) in {0, >=1024*}
      * masked row max/min: VectorE tensor_tensor min/max + tensor_reduce,
        all in squared-distance space (sqrt deferred to [128, 8] tail)
      * tail: sqrt, relu thresholds, partition-sum via ones-matmul
  - Host: sum the 8 per-core [2,1] partials, divide by N (the "all-reduce").

The [8192, 8192] distance matrices never leave PSUM.
"""

import numpy as np

N_SEL = 8192
N_CORES = 8
ROWS_PER_CORE = N_SEL // N_CORES  # 1024
M_TILES = ROWS_PER_CORE // 128  # 8
NT = 512
N_TILES = N_SEL // NT  # 16
K_EXT = 69  # feats ext block at partitions 0:34, pts ext block at 64:69

EPS = 1e-7
POS_RADIUS = 0.0375
NEG_RADIUS = 0.1
POS_THRESH = 0.1
NEG_THRESH = 1.4
C1 = float(np.float32(POS_RADIUS**2 - EPS))  # pos: pd2 < C1
C2 = float(np.float32(NEG_RADIUS**2 - EPS))  # neg: pd2 > C2
S = 1.0e13

_PROGRAM_CACHE: dict = {}
KERNEL_CFG = {"red_engine": "dve", "sel_dtype": "f32"}


def build_program(repeat: int = 1, red_engine: str = "dve",
                  sel_dtype: str = "f32", group: int = 1):
    """Build the Bass program (one NeuronCore, run SPMD on 8)."""
    import concourse.bacc as bacc
    import concourse.mybir as mybir
    import concourse.tile as tile

    f32 = mybir.dt.float32
    seldt = mybir.dt.float32 if sel_dtype == "f32" else mybir.dt.bfloat16
    A = mybir.AluOpType
    AF = mybir.ActivationFunctionType
    X = mybir.AxisListType.X

    nc = bacc.Bacc("TRN2", target_bir_lowering=False, debug=False,
                   num_devices=N_CORES)
    srcT_d = nc.dram_tensor("srcT", [K_EXT, ROWS_PER_CORE], f32,
                            kind="ExternalInput").ap()
    tgtT_d = nc.dram_tensor("tgtT", [K_EXT, N_SEL], f32,
                            kind="ExternalInput").ap()
    rtt_d = nc.dram_tensor("rtt", [3, 4], f32, kind="ExternalInput").ap()
    out_d = nc.dram_tensor("out", [2, 1], f32, kind="ExternalOutput").ap()

    with tile.TileContext(nc) as tc:
        with (
            tc.tile_pool(name="big", bufs=1) as big,
            tc.tile_pool(name="mask", bufs=4) as mask_p,
            tc.tile_pool(name="val", bufs=4) as val_p,
            tc.tile_pool(name="red", bufs=2) as red_p,
            tc.tile_pool(name="small", bufs=4) as small,
            tc.tile_pool(name="pf", bufs=(3 if group == 1 else 2),
                         space="PSUM") as pf_p,
            tc.tile_pool(name="pp", bufs=(3 if group == 1 else 2),
                         space="PSUM") as pp_p,
        ):
            rhs = big.tile([K_EXT, N_SEL], f32, tag="rhs")
            lhs = big.tile([K_EXT, ROWS_PER_CORE], f32, tag="lhs")
            rtt_sb = big.tile([3, 4], f32, tag="rtt")
            praw = big.tile([3, ROWS_PER_CORE], f32, tag="praw")
            prot = big.tile([3, ROWS_PER_CORE], f32, tag="prot")
            sqt = big.tile([K_EXT, N_SEL], f32, tag="sqt")
            sqs = big.tile([K_EXT, ROWS_PER_CORE], f32, tag="sqs")
            nlT = big.tile([K_EXT, 2], f32, tag="nlT")
            nlS = big.tile([K_EXT, 2], f32, tag="nlS")
            ones128 = big.tile([128, 1], f32, tag="ones128")
            fp2all = big.tile([128, M_TILES], f32, tag="fp2all")
            cn2all = big.tile([128, M_TILES], f32, tag="cn2all")
            accT = big.tile([128, 2], f32, tag="accT")
            b1 = big.tile([128, 1], f32, tag="b1")
            b2 = big.tile([128, 1], f32, tag="b2")
            beps = big.tile([128, 1], f32, tag="beps")
            bpos = big.tile([128, 1], f32, tag="bpos")
            bneg = big.tile([128, 1], f32, tag="bneg")

            nc.sync.dma_start(rhs[:], tgtT_d[:])
            nc.sync.dma_start(lhs[:], srcT_d[:])
            nc.sync.dma_start(rtt_sb[:], rtt_d[:])
            nc.gpsimd.memset(ones128[:], 1.0)
            nc.gpsimd.memset(b1[:], float(np.float32(S * C1)))
            nc.gpsimd.memset(b2[:], float(np.float32(S * C2)))
            nc.gpsimd.memset(beps[:], EPS)
            nc.gpsimd.memset(bpos[:], -POS_THRESH)
            nc.gpsimd.memset(bneg[:], NEG_THRESH)
            nc.sync.dma_start(praw[:], srcT_d[64:67, :])
            nc.gpsimd.memset(nlT[:], 0.0)
            nc.gpsimd.memset(nlT[0:32, 0:1], 1.0)
            nc.gpsimd.memset(nlT[64:67, 1:2], 1.0)
            nc.gpsimd.memset(nlS[:], 0.0)
            nc.gpsimd.memset(nlS[0:32, 0:1], 0.25)
            nc.gpsimd.memset(nlS[64:67, 1:2], 0.25)

            # ---- src-side prep: lhs rows [-2sf |1| nsf | -2sp' |1| nsp'] ----
            nc.scalar.activation(lhs[0:32, :], lhs[0:32, :], AF.Copy,
                                 bias=0.0, scale=-2.0)
            for ch in range(ROWS_PER_CORE // NT):
                sl = slice(ch * NT, (ch + 1) * NT)
                psr = pf_p.tile([3, NT], f32, tag="psf")
                nc.tensor.matmul(out=psr[:], lhsT=rtt_sb[0:3, 0:3],
                                 rhs=praw[:, sl], start=True, stop=True)
                # -2 * (R p + t)
                nc.vector.tensor_scalar(
                    out=prot[:, sl], in0=psr[:],
                    scalar1=rtt_sb[0:3, 3:4], scalar2=-2.0,
                    op0=A.add, op1=A.mult)
            # move rotated pts into the pts ext block (partition shift -> DMA)
            nc.sync.dma_start(lhs[64:67, :], prot[:])
            nc.scalar.activation(sqs[:], lhs[0:K_EXT, :], AF.Square)
            for ch in range(ROWS_PER_CORE // NT):
                sl = slice(ch * NT, (ch + 1) * NT)
                psn = pf_p.tile([2, NT], f32, tag="psf")
                nc.tensor.matmul(out=psn[:], lhsT=nlS[:], rhs=sqs[:, sl],
                                 start=True, stop=True)
                stg = small.tile([2, NT], f32, tag="stg")
                nc.scalar.copy(stg[:], psn[:])
                nc.sync.dma_start(lhs[33:34, sl], stg[0:1, :])
                nc.sync.dma_start(lhs[68:69, sl], stg[1:2, :])

            # ---- tgt-side prep: rhs rows [tf | ntf |1| tp | ntp |1] ----
            nc.scalar.activation(sqt[:], rhs[0:K_EXT, :], AF.Square)
            for ch in range(N_TILES):
                sl = slice(ch * NT, (ch + 1) * NT)
                psn = pf_p.tile([2, NT], f32, tag="psf")
                nc.tensor.matmul(out=psn[:], lhsT=nlT[:], rhs=sqt[:, sl],
                                 start=True, stop=True)
                stg = small.tile([2, NT], f32, tag="stg")
                nc.scalar.copy(stg[:], psn[:])
                nc.sync.dma_start(rhs[32:33, sl], stg[0:1, :])
                nc.sync.dma_start(rhs[67:68, sl], stg[1:2, :])

            red = nc.vector if red_engine == "dve" else nc.gpsimd

            GNT = NT * group
            GN_TILES = N_SEL // GNT

            def main_loop(_iv=None):
                for m in range(M_TILES):
                    msl = slice(m * 128, (m + 1) * 128)
                    pos_r = red_p.tile([128, GN_TILES], f32, tag="pos_r")
                    neg_r = red_p.tile([128, GN_TILES], f32, tag="neg_r")
                    for n in range(GN_TILES):
                        psf = pf_p.tile([128, GNT], f32, tag="psf")
                        psp = pp_p.tile([128, GNT], f32, tag="psp")
                        for g in range(group):
                            nsl = slice(n * GNT + g * NT,
                                        n * GNT + (g + 1) * NT)
                            gsl = slice(g * NT, (g + 1) * NT)
                            nc.tensor.matmul(out=psf[:, gsl],
                                             lhsT=lhs[0:34, msl],
                                             rhs=rhs[0:34, nsl],
                                             start=True, stop=True)
                            nc.tensor.matmul(out=psp[:, gsl],
                                             lhsT=lhs[64:69, msl],
                                             rhs=rhs[64:69, nsl],
                                             start=True, stop=True)
                        t1 = mask_p.tile([128, GNT], seldt, tag="t1")
                        nc.scalar.activation(t1[:], psp[:], AF.Relu,
                                             bias=b1[:], scale=-S)
                        t2 = mask_p.tile([128, GNT], seldt, tag="t2")
                        nc.scalar.activation(t2[:], psp[:], AF.Relu,
                                             bias=b2[:], scale=-S)
                        if sel_dtype == "bf16":
                            fsel = val_p.tile([128, GNT], seldt, tag="fsel")
                            nc.vector.tensor_copy(fsel[:], psf[:])
                        else:
                            fsel = psf
                        posv = val_p.tile([128, GNT], seldt, tag="posv")
                        nc.vector.tensor_tensor(out=posv[:], in0=fsel[:],
                                                in1=t1[:], op=A.min)
                        negv = val_p.tile([128, GNT], seldt, tag="negv")
                        nc.vector.tensor_tensor(out=negv[:], in0=fsel[:],
                                                in1=t2[:], op=A.max)
                        red.tensor_reduce(out=pos_r[:, n:n + 1], in_=posv[:],
                                          op=A.max, axis=X)
                        red.tensor_reduce(out=neg_r[:, n:n + 1], in_=negv[:],
                                          op=A.min, axis=X)
                    nc.vector.tensor_reduce(out=fp2all[:, m:m + 1],
                                            in_=pos_r[:], op=A.max, axis=X)
                    nc.vector.tensor_reduce(out=cn2all[:, m:m + 1],
                                            in_=neg_r[:], op=A.min, axis=X)

            if repeat == 1:
                main_loop()
            else:
                with tc.For_i(0, repeat, 1) as iv:
                    main_loop(iv)

            # ---- tail: sqrt / relu thresholds / partition sums ----
            fp = small.tile([128, M_TILES], f32, tag="fp")
            cn = small.tile([128, M_TILES], f32, tag="cn")
            nc.scalar.activation(fp[:], fp2all[:], AF.Sqrt, bias=beps[:])
            nc.scalar.activation(cn[:], cn2all[:], AF.Sqrt, bias=beps[:])
            pl = small.tile([128, M_TILES], f32, tag="pl")
            nl = small.tile([128, M_TILES], f32, tag="nl")
            nc.scalar.activation(pl[:], fp[:], AF.Relu, bias=bpos[:])
            nc.scalar.activation(nl[:], cn[:], AF.Relu, bias=bneg[:],
                                 scale=-1.0)
            nc.vector.tensor_reduce(out=accT[:, 0:1], in_=pl[:], op=A.add,
                                    axis=X)
            nc.vector.tensor_reduce(out=accT[:, 1:2], in_=nl[:], op=A.add,
                                    axis=X)
            pso = pf_p.tile([2, 1], f32, tag="psf")
            nc.tensor.matmul(out=pso[:], lhsT=accT[:], rhs=ones128[:],
                             start=True, stop=True)
            res_sb = small.tile([2, 1], f32, tag="res")
            nc.scalar.copy(res_sb[:], pso[:])
            nc.sync.dma_start(out_d[:], res_sb[:])

    nc.compile()
    return nc


def make_in_maps(src_pcd, tgt_pcd, src_feats, tgt_feats, correspondence,
                 rot, trans):
    """Host-side gather/shard/layout (indexing + transpose only)."""
    ci = np.asarray(correspondence[:, 0]).astype(np.int64)
    cj = np.asarray(correspondence[:, 1]).astype(np.int64)
    src_pcd = np.asarray(src_pcd, np.float32)
    tgt_pcd = np.asarray(tgt_pcd, np.float32)
    src_feats = np.asarray(src_feats, np.float32)
    tgt_feats = np.asarray(tgt_feats, np.float32)

    tgtT = np.zeros((K_EXT, N_SEL), np.float32)
    tgtT[0:32] = tgt_feats[cj].T
    tgtT[33] = 1.0
    tgtT[64:67] = tgt_pcd[cj].T
    tgtT[68] = 1.0

    srcT = np.zeros((K_EXT, N_SEL), np.float32)
    srcT[0:32] = src_feats[ci].T  # device scales by -2
    srcT[32] = 1.0
    srcT[64:67] = src_pcd[ci].T  # device applies rot/trans and -2
    srcT[67] = 1.0

    rtt = np.zeros((3, 4), np.float32)
    rtt[:, 0:3] = np.asarray(rot, np.float32).T
    rtt[:, 3] = np.asarray(trans, np.float32)[:, 0]

    in_maps = []
    for c in range(N_CORES):
        sl = slice(c * ROWS_PER_CORE, (c + 1) * ROWS_PER_CORE)
        in_maps.append({
            "srcT": np.ascontiguousarray(srcT[:, sl]),
            "tgtT": tgtT,
            "rtt": rtt,
        })
    return in_maps


def combine_outputs(results):
    """Host-side unshard: sum per-core partial sums, divide by N."""
    tot = np.zeros(2, np.float32)
    for r in results:
        tot += r["out"][:, 0].astype(np.float32)
    loss = np.float32(tot[0] / np.float32(N_SEL) + tot[1] / np.float32(N_SEL))
    return np.float32(loss)


def kernel(src_pcd, tgt_pcd, src_feats, tgt_feats, correspondence, rot,
           trans):
    from concourse import bass_utils

    key = ("prog", 1, KERNEL_CFG["red_engine"], KERNEL_CFG["sel_dtype"])
    if key not in _PROGRAM_CACHE:
        _PROGRAM_CACHE[key] = build_program(
            repeat=1, red_engine=KERNEL_CFG["red_engine"],
            sel_dtype=KERNEL_CFG["sel_dtype"])
    nc = _PROGRAM_CACHE[key]
    in_maps = make_in_maps(src_pcd, tgt_pcd, src_feats, tgt_feats,
                           correspondence, rot, trans)
    res = bass_utils.run_bass_kernel_spmd(nc, in_maps,
                                          core_ids=list(range(N_CORES)))
    return combine_outputs(res.results)



# revision 33
# speedup vs baseline: 5.7118x; 5.7118x over previous
"""Trainium2 Bass kernel for nn_HardestContrastiveLoss.

Strategy (1D row-parallel cdist, per sharding hint):
  - Host: gather the selected correspondences (pure indexing/layout), build
    transposed operand blocks, shard 8192 selected rows as 1024 rows/core.
  - Device (per core, identical program, different data):
      * prep: rigid-transform gathered src points (small matmul + fused
        add/scale), scale feats by -2, one square pass + ones-matmul for
        the norm rows -> extended matmul operands
      * two fp32r matmuls per [128, 512] tile (1 cycle/row on TRN2 for
        moving dim >= 256, vs 4 for plain fp32):
          psf  = |a-b|^2 (feats)      via [-2a | n | 1] . [b | 1 | n]
          psp1 = V*(C1 - |p-q|^2) + d via threshold-folded pts GEMM
        With V=1e13 the fp32 accumulation quantizes psp1 to multiples of
        ~32768 (plus d = |a|^2+|b|^2 <= ~340), so every "pos/neg gap" is
        far larger than any feats distance^2: a single elementwise min/max
        against the feats distances is an exact mask-select -- no ScalarE
        mask passes; the neg threshold is psp1 + D, D = V*(C2-C1).
      * per [128, 1024] macro tile: ScalarE stages psf and psp1 to SBUF
        as bf16 (PSUM-port limits keep 2-input DVE ops at 1x from PSUM;
        all-bf16 SBUF operands unlock the DVE 2x_1P packed mode);
        selects + row-reduces all on DVE:
          pos: reduce_max(min(psp1, fd2))            [tensor_tensor]
          neg: reduce_min(max(psp1 + D, fd2))        [scalar_tensor_tensor]
      * tail: clamp, sqrt, relu thresholds, partition-sum via ones-matmul
  - Host: sum the 8 per-core [2,1] partials, divide by N (the "all-reduce").

The [8192, 8192] distance matrices never leave PSUM.

Operand row layout ([40, .] blocks; PE requires operand base partition in
{0, 32, 64}, both operands equal):
  src (lhsT)                 tgt (rhs)
  0:32  -2a                  b
  32    |a|^2 (dev)          1
  33    1                    |b|^2 (dev)
  34:37 2V*(R p + t) (dev)   q
  37    -V|p^|^2 (dev)       1
  38    -V                   |q|^2 (dev)
  39    1                    V*C1
  feats matmul = [0:34] x [0:34];  psp1 = [32:40] x [32:40] (base 32)
"""

import numpy as np

N_SEL = 8192
N_CORES = 8
ROWS_PER_CORE = N_SEL // N_CORES  # 1024
M_TILES = ROWS_PER_CORE // 128  # 8
NT = 512  # matmul tile (one PSUM bank)
GNT = 1024  # DVE macro tile (two PSUM banks)
GN_TILES = N_SEL // GNT  # 8
K_TOT = 40

EPS = 1e-7
POS_RADIUS = 0.0375
NEG_RADIUS = 0.1
POS_THRESH = 0.1
NEG_THRESH = 1.4
C1 = float(np.float32(POS_RADIUS**2 - EPS))  # pos: pd2 < C1
C2 = float(np.float32(NEG_RADIUS**2 - EPS))  # neg: pd2 > C2
V = 1.0e13  # threshold-fold scale; fp32 ulp at V*C2 magnitude >> max fd2
DSHIFT = float(np.float32(V) * np.float32(C2) - np.float32(V) * np.float32(C1))

_PROGRAM_CACHE: dict = {}


def build_program(repeat: int = 1):
    """Build the Bass program (one NeuronCore, run SPMD on 8)."""
    import concourse.bacc as bacc
    import concourse.mybir as mybir
    import concourse.tile as tile

    f32 = mybir.dt.float32
    f32r = mybir.dt.float32r
    bf16 = mybir.dt.bfloat16
    A = mybir.AluOpType
    AF = mybir.ActivationFunctionType
    X = mybir.AxisListType.X

    nc = bacc.Bacc("TRN2", target_bir_lowering=False, debug=False,
                   num_devices=N_CORES)
    srcT_d = nc.dram_tensor("srcT", [K_TOT, ROWS_PER_CORE], f32,
                            kind="ExternalInput").ap()
    tgtT_d = nc.dram_tensor("tgtT", [K_TOT, N_SEL], f32,
                            kind="ExternalInput").ap()
    rtt_d = nc.dram_tensor("rtt", [3, 4], f32, kind="ExternalInput").ap()
    cst_d = nc.dram_tensor("cst", [K_TOT, 4], f32,
                           kind="ExternalInput").ap()
    out_d = nc.dram_tensor("out", [2, 1], f32, kind="ExternalOutput").ap()

    with tile.TileContext(nc) as tc:
        with (
            tc.tile_pool(name="big", bufs=1) as big,
            tc.tile_pool(name="scr", bufs=4) as scr_p,
            tc.tile_pool(name="red", bufs=2) as red_p,
            tc.tile_pool(name="small", bufs=4) as small,
            tc.tile_pool(name="pf", bufs=2, space="PSUM") as pf_p,
            tc.tile_pool(name="pp1", bufs=2, space="PSUM") as pp1_p,
        ):
            rhs = big.tile([K_TOT, N_SEL], f32r, tag="rhs")
            lhs = big.tile([K_TOT, ROWS_PER_CORE], f32r, tag="lhs")
            rtt_sb = big.tile([3, 4], f32, tag="rtt")
            praw = big.tile([3, ROWS_PER_CORE], f32, tag="praw")
            prot = big.tile([3, ROWS_PER_CORE], f32, tag="prot")
            sqt = big.tile([K_TOT, N_SEL], f32r, tag="sqt")
            sqs = big.tile([K_TOT, ROWS_PER_CORE], f32r, tag="sqs")
            nlW = big.tile([K_TOT, 4], f32r, tag="nlW")
            ones128 = big.tile([128, 1], f32, tag="ones128")
            fp2all = big.tile([128, M_TILES], f32, tag="fp2all")
            cn2all = big.tile([128, M_TILES], f32, tag="cn2all")
            accT = big.tile([128, 2], f32, tag="accT")
            beps = big.tile([128, 1], f32, tag="beps")
            bpos = big.tile([128, 1], f32, tag="bpos")
            bneg = big.tile([128, 1], f32, tag="bneg")

            nc.sync.dma_start(lhs[:], srcT_d.bitcast(f32r)[0:K_TOT, :])
            nc.sync.dma_start(rtt_sb[:], rtt_d[:])
            nc.sync.dma_start(praw[:], srcT_d[34:37, :])
            nc.sync.dma_start(nlW[:], cst_d.bitcast(f32r)[:])
            nc.gpsimd.memset(ones128[:], 1.0)
            nc.gpsimd.memset(beps[:], EPS)
            nc.gpsimd.memset(bpos[:], -POS_THRESH)
            nc.gpsimd.memset(bneg[:], NEG_THRESH)

            # ---- src-side prep ----
            nc.scalar.activation(lhs[0:32, :], lhs.bitcast(f32)[0:32, :], AF.Copy,
                                 bias=0.0, scale=-2.0)
            for ch in range(ROWS_PER_CORE // NT):
                sl = slice(ch * NT, (ch + 1) * NT)
                psrt = pf_p.tile([128, GNT], f32, tag="psf")
                psr = psrt[0:3, 0:NT]
                nc.tensor.matmul(out=psr, lhsT=rtt_sb[0:3, 0:3],
                                 rhs=praw[:, sl], start=True, stop=True)
                # 2V * (R p + t)
                nc.vector.tensor_scalar(
                    out=prot[:, sl], in0=psr,
                    scalar1=rtt_sb[0:3, 3:4], scalar2=2.0 * V,
                    op0=A.add, op1=A.mult)
            # rotated pts into the pts block (partition shift -> DMA)
            nc.sync.dma_start(lhs[34:37, :], prot.bitcast(f32r)[:])
            nc.scalar.activation(sqs[:], lhs.bitcast(f32)[:], AF.Square)
            for ch in range(ROWS_PER_CORE // NT):
                sl = slice(ch * NT, (ch + 1) * NT)
                psnt = pf_p.tile([128, GNT], f32, tag="psf")
                psn = psnt[0:2, 0:NT]
                nc.tensor.matmul(out=psn, lhsT=nlW[:, 0:2],
                                 rhs=sqs[:, sl],
                                 start=True, stop=True)
                stg = small.tile([2, NT], f32, tag="stg")
                nc.scalar.copy(stg[:], psn)
                nc.sync.dma_start(lhs[32:33, sl], stg.bitcast(f32r)[0:1, :])
                nc.sync.dma_start(lhs[37:38, sl], stg.bitcast(f32r)[1:2, :])

            # ---- tgt-side prep: one 512-col chunk (DMA -> square ->
            # norm rows); emitted interleaved with the m=0 main pass so the
            # PE/Act in-order queues pipeline prep under early macros ----
            def tgt_prep_chunk(ch):
                sl = slice(ch * NT, (ch + 1) * NT)
                nc.sync.dma_start(rhs[:, sl], tgtT_d.bitcast(f32r)[:, sl])
                nc.scalar.activation(sqt[:, sl], rhs.bitcast(f32)[:, sl],
                                     AF.Square)
                psnt = pf_p.tile([128, GNT], f32, tag="psf")
                psn = psnt[0:2, 0:NT]
                nc.tensor.matmul(out=psn, lhsT=nlW[:, 2:4],
                                 rhs=sqt[:, sl],
                                 start=True, stop=True)
                stg = small.tile([2, NT], f32, tag="stg")
                nc.scalar.copy(stg[:], psn)
                nc.sync.dma_start(rhs[33:34, sl], stg.bitcast(f32r)[0:1, :])
                nc.sync.dma_start(rhs[38:39, sl], stg.bitcast(f32r)[1:2, :])

            tgt_prep_chunk(0)
            tgt_prep_chunk(1)

            def main_loop(_iv=None):
                for m in range(M_TILES):
                    msl = slice(m * 128, (m + 1) * 128)
                    pos_r = red_p.tile([128, GN_TILES // 4], f32,
                                       tag="pos_r")
                    neg_r = red_p.tile([128, GN_TILES // 4], f32,
                                       tag="neg_r")
                    stage = []
                    for n in range(GN_TILES):
                        if repeat == 1 and m == 0 and n < GN_TILES - 1:
                            tgt_prep_chunk(2 * n + 2)
                            tgt_prep_chunk(2 * n + 3)
                        psf = pf_p.tile([128, GNT], f32, tag="psf")
                        psp1 = pp1_p.tile([128, GNT], f32, tag="psp1")
                        for g in range(2):
                            nsl = slice(n * GNT + g * NT,
                                        n * GNT + (g + 1) * NT)
                            gsl = slice(g * NT, (g + 1) * NT)
                            nc.tensor.matmul(out=psf[:, gsl],
                                             lhsT=lhs[0:34, msl],
                                             rhs=rhs[0:34, nsl],
                                             start=True, stop=True)
                            nc.tensor.matmul(out=psp1[:, gsl],
                                             lhsT=lhs[32:40, msl],
                                             rhs=rhs[32:40, nsl],
                                             start=True, stop=True)
                        fsb = scr_p.tile([128, GNT], bf16, tag="fsb")
                        nc.scalar.copy(fsb[:], psf[:])
                        psb1 = scr_p.tile([128, GNT], bf16, tag="psb1")
                        nc.scalar.copy(psb1[:], psp1[:])
                        scr1 = scr_p.tile([128, GNT], bf16, tag="scr1")
                        nc.vector.tensor_tensor(out=scr1[:], in0=psb1[:],
                                                in1=fsb[:], op=A.min)
                        scr2 = scr_p.tile([128, GNT], bf16, tag="scr2")
                        # stage the D-shifted mask for a 2x-mode bf16 TT:
                        # ~40% on Act (copy+bias from PSUM), the rest on DVE
                        # (4x-mode bf16 tensor_scalar from psb1)
                        psb1D = scr_p.tile([128, GNT], bf16, tag="psb1D")
                        if n % 5 < 2:
                            nc.scalar.activation(psb1D[:], psp1[:], AF.Copy,
                                                 bias=DSHIFT, scale=1.0)
                        else:
                            nc.vector.tensor_scalar(
                                out=psb1D[:], in0=psb1[:], scalar1=DSHIFT,
                                scalar2=None, op0=A.add)
                        nc.vector.tensor_tensor(out=scr2[:],
                                                in0=psb1D[:],
                                                in1=fsb[:], op=A.max)
                        stage.append((scr1, scr2))
                        if len(stage) == 4:
                            # fold 4 select outputs, then one reduce per side
                            c1a = scr_p.tile([128, GNT], bf16, tag="c1a")
                            nc.vector.tensor_tensor(out=c1a[:],
                                                    in0=stage[0][0][:],
                                                    in1=stage[1][0][:],
                                                    op=A.max)
                            c1b = scr_p.tile([128, GNT], bf16, tag="c1b")
                            nc.vector.tensor_tensor(out=c1b[:],
                                                    in0=stage[2][0][:],
                                                    in1=stage[3][0][:],
                                                    op=A.max)
                            c1 = scr_p.tile([128, GNT], bf16, tag="c1")
                            nc.vector.tensor_tensor(out=c1[:], in0=c1a[:],
                                                    in1=c1b[:], op=A.max)
                            c2a = scr_p.tile([128, GNT], bf16, tag="c2a")
                            nc.vector.tensor_tensor(out=c2a[:],
                                                    in0=stage[0][1][:],
                                                    in1=stage[1][1][:],
                                                    op=A.min)
                            c2b = scr_p.tile([128, GNT], bf16, tag="c2b")
                            nc.vector.tensor_tensor(out=c2b[:],
                                                    in0=stage[2][1][:],
                                                    in1=stage[3][1][:],
                                                    op=A.min)
                            c2 = scr_p.tile([128, GNT], bf16, tag="c2")
                            nc.vector.tensor_tensor(out=c2[:], in0=c2a[:],
                                                    in1=c2b[:], op=A.min)
                            k = n // 4
                            nc.vector.tensor_reduce(out=pos_r[:, k:k + 1],
                                                    in_=c1[:], op=A.max,
                                                    axis=X)
                            nc.vector.tensor_reduce(out=neg_r[:, k:k + 1],
                                                    in_=c2[:], op=A.min,
                                                    axis=X)
                            stage.clear()
                    nc.vector.tensor_reduce(out=fp2all[:, m:m + 1],
                                            in_=pos_r[:], op=A.max, axis=X)
                    nc.vector.tensor_reduce(out=cn2all[:, m:m + 1],
                                            in_=neg_r[:], op=A.min, axis=X)

            if repeat == 1:
                main_loop()
            else:
                for ch in range(2, N_SEL // NT):
                    tgt_prep_chunk(ch)
                with tc.For_i(0, repeat, 1) as iv:
                    main_loop(iv)

            # ---- tail: sqrt / relu thresholds / partition sums ----
            fp2c = small.tile([128, M_TILES], f32, tag="fp2c")
            nc.scalar.activation(fp2c[:], fp2all[:], AF.Relu)
            fp = small.tile([128, M_TILES], f32, tag="fp")
            cn = small.tile([128, M_TILES], f32, tag="cn")
            nc.scalar.activation(fp[:], fp2c[:], AF.Sqrt, bias=beps[:])
            nc.scalar.activation(cn[:], cn2all[:], AF.Sqrt, bias=beps[:])
            pl = small.tile([128, M_TILES], f32, tag="pl")
            nl = small.tile([128, M_TILES], f32, tag="nl")
            nc.scalar.activation(pl[:], fp[:], AF.Relu, bias=bpos[:])
            nc.scalar.activation(nl[:], cn[:], AF.Relu, bias=bneg[:],
                                 scale=-1.0)
            nc.vector.tensor_reduce(out=accT[:, 0:1], in_=pl[:], op=A.add,
                                    axis=X)
            nc.vector.tensor_reduce(out=accT[:, 1:2], in_=nl[:], op=A.add,
                                    axis=X)
            psot = pf_p.tile([128, GNT], f32, tag="psf")
            pso = psot[0:2, 0:1]
            nc.tensor.matmul(out=pso, lhsT=accT[:], rhs=ones128[:],
                             start=True, stop=True)
            res_sb = small.tile([2, 1], f32, tag="res")
            nc.scalar.copy(res_sb[:], pso)
            nc.sync.dma_start(out_d[:], res_sb[:])

    nc.compile()
    return nc


def make_in_maps(src_pcd, tgt_pcd, src_feats, tgt_feats, correspondence,
                 rot, trans):
    """Host-side gather/shard/layout (indexing + constant fills only)."""
    ci = np.asarray(correspondence[:, 0]).astype(np.int64)
    cj = np.asarray(correspondence[:, 1]).astype(np.int64)
    src_pcd = np.asarray(src_pcd, np.float32)
    tgt_pcd = np.asarray(tgt_pcd, np.float32)
    src_feats = np.asarray(src_feats, np.float32)
    tgt_feats = np.asarray(tgt_feats, np.float32)

    # center pts at the box center: tf32 (fp32r) input rounding error is
    # relative to coordinate magnitude; |p-q|^2 is shift-invariant
    CEN = np.float32(0.1)

    tgtT = np.zeros((K_TOT, N_SEL), np.float32)
    tgtT[0:32] = tgt_feats[cj].T
    tgtT[32] = 1.0
    # [33] = |b|^2 (device)
    tgtT[34:37] = tgt_pcd[cj].T - CEN
    tgtT[37] = 1.0
    # [38] = |q|^2 (device)
    tgtT[39] = np.float32(V) * np.float32(C1)

    srcT = np.zeros((K_TOT, N_SEL), np.float32)
    srcT[0:32] = src_feats[ci].T  # device scales by -2
    # [32] = |a|^2 (device)
    srcT[33] = 1.0
    srcT[34:37] = src_pcd[ci].T  # device applies rot/trans and 2V
    # [37] = -V|p^|^2 (device)
    srcT[38] = -np.float32(V)
    srcT[39] = 1.0

    rtt = np.zeros((3, 4), np.float32)
    rtt[:, 0:3] = np.asarray(rot, np.float32).T
    rtt[:, 3] = np.asarray(trans, np.float32)[:, 0] - CEN

    # norm-row matmul weights: col0/1 src-side, col2/3 tgt-side
    cst = np.zeros((K_TOT, 4), np.float32)
    cst[0:32, 0] = 0.25                          # |a|^2 from (-2a)^2
    cst[34:37, 1] = np.float32(-1.0 / (4.0 * V))  # -V|p^|^2 from (2Vp^)^2
    cst[0:32, 2] = 1.0                           # |b|^2 from b^2
    cst[34:37, 3] = 1.0                          # |q|^2 from q^2

    in_maps = []
    for c in range(N_CORES):
        sl = slice(c * ROWS_PER_CORE, (c + 1) * ROWS_PER_CORE)
        in_maps.append({
            "srcT": np.ascontiguousarray(srcT[:, sl]),
            "tgtT": tgtT,
            "rtt": rtt,
            "cst": cst,
        })
    return in_maps


def combine_outputs(results):
    """Host-side unshard: sum per-core partial sums, divide by N."""
    tot = np.zeros(2, np.float32)
    for r in results:
        tot += r["out"][:, 0].astype(np.float32)
    loss = np.float32(tot[0] / np.float32(N_SEL) + tot[1] / np.float32(N_SEL))
    return np.float32(loss)


def kernel(src_pcd, tgt_pcd, src_feats, tgt_feats, correspondence, rot,
           trans):
    from concourse import bass_utils

    key = ("prog", 1)
    if key not in _PROGRAM_CACHE:
        _PROGRAM_CACHE[key] = build_program(repeat=1)
    nc = _PROGRAM_CACHE[key]
    in_maps = make_in_maps(src_pcd, tgt_pcd, src_feats, tgt_feats,
                           correspondence, rot, trans)
    res = bass_utils.run_bass_kernel_spmd(nc, in_maps,
                                          core_ids=list(range(N_CORES)))
    return combine_outputs(res.results)
